# revision 15
# baseline (speedup 1.0000x reference)
"""Trainium2 Bass kernel for the soft-DTW shape+temporal loss.

Problem: input/target (4, 128, 16, 4, 4) = (B, T, C, H, W). Each of the
B*C*H*W = 1024 spatial cells is an independent univariate series of length
T = 128. Per series: squared-L2 cost matrix D, soft-DTW forward DP value
R[N,N] (loss_shape), soft alignment path = dR[N,N]/dD (via the standard
backward E-recursion), temporal loss = sum(path * Omega)/T^2 with
Omega[i,j] = (i-j)^2.

Sharding: 1024 series / 8 cores = 128 series per core, one series per SBUF
partition. The DP runs along anti-diagonals (wavefront); diagonal cells are
laid out along the free dimension, so every wavefront step is a handful of
full-width vector ops. R is stored diagonal-major (slot width DL per
diagonal) for the whole forward sweep; the backward pass re-reads it and
recomputes D diagonals on the fly from sentinel-padded series.

Host side shards the inputs, runs the same program SPMD on 8 cores, and
reduces the per-series results to the 3 scalar losses.
"""

import sys

for _p in ("/opt/trn_rl_repo",):
    if _p not in sys.path:
        sys.path.insert(0, _p)

import numpy as np

import concourse.bass as bass
import concourse.mybir as mybir
from concourse import bass_utils
from concourse.tile import TileContext

# ---- problem constants (hardcoded per contract) ----
B, T, C, H, W = 4, 128, 16, 4, 4
N = T
NCORES = 8
SPC = (B * C * H * W) // NCORES  # 128 series per core
ALPHA = 0.5
GAMMA = 0.01
INVG = 1.0 / GAMMA
BIG = 1e8
SENT = 1.0e6  # sentinel pad; (x - SENT)^2 ~ 1e12 >> BIG kills boundary weights

DL = N + 4  # per-diagonal slot width (positions 0..N+1 used)
ND = 2 * N + 1  # diagonals 0..2N
KS = 4  # S ring depth
KE = 4  # E ring depth
SQOFF = 256  # sqtab column offset: col = (2*idx - d) + SQOFF

F32 = mybir.dt.float32
I32 = mybir.dt.int32
AF = mybir.ActivationFunctionType
OP = mybir.AluOpType


def _rng(d):
    """Valid idx range [lo, hi] of diagonal d (cells (i=idx, j=d-idx))."""
    return max(1, d - N), min(N, d - 1)


def _split_multi_waits(nc):
    """walrus here rejects >1 sync wait per TPB instruction.

    Pass 1: drop self-engine waits that are provably satisfied by program
    order (the engine already issued >= wait_value increments of its own
    semaphore earlier in the stream) — Tile emits these for WAR hygiene but
    they are redundant on an in-order engine.
    Pass 2: hoist remaining extra waits onto same-engine NoOp carriers
    (legal on DVE/SP/Pool here; ACT NoOp is rejected by codegen, but pass 1
    leaves ACT instructions with at most one wait).
    """
    pre_of = {
        mybir.EngineType.DVE: "DVE",
        mybir.EngineType.Activation: "Activation",
        mybir.EngineType.Pool: "Pool",
        mybir.EngineType.SP: "SP",
        mybir.EngineType.PE: "PE",
    }
    nsplit = 0
    inc = {}  # (engine, sem id) -> inc count so far, in block order
    tainted = set()
    for f in nc.m.functions:
        for bb in f.blocks:
            insts = list(bb.instructions)
            new = []
            changed = False
            for ins in insts:
                si = ins.sync_info
                eng = ins.engine
                pre = pre_of.get(eng)
                waits = list(si.on_wait) if si is not None and si.on_wait else []
                if (
                    waits
                    and pre is not None
                    and len(waits) > 1
                    and eng == mybir.EngineType.Activation
                ):
                    keep = [
                        w
                        for w in waits
                        if not (
                            w.sync_type == "semaphore"
                            and w.wait_mode == "sem-ge-imm"
                            and w.ant_name
                            and w.ant_name.split("_")[0] == pre
                            and w.id not in tainted
                            and w.wait_value <= inc.get((eng, w.id), 0)
                        )
                    ]
                else:
                    keep = waits
                if len(keep) > 1:
                    for w in keep[:-1]:
                        nsplit += 1
                        new.append(
                            mybir.InstNoOp(
                                name=f"wsplit-{nsplit}",
                                engine=eng,
                                sync_info=mybir.SyncInfo(on_wait=[w], on_update=[]),
                            )
                        )
                    keep = [keep[-1]]
                    changed = True
                if si is not None and len(keep) != len(waits):
                    ins.sync_info = mybir.SyncInfo(
                        on_wait=keep, on_update=list(si.on_update or [])
                    )
                    changed = True
                if si is not None and si.on_update:
                    for u in si.on_update:
                        if u.update_mode == "sem-inc":
                            inc[(eng, u.id)] = inc.get((eng, u.id), 0) + (
                                u.update_value or 0
                            )
                        else:
                            tainted.add(u.id)
                new.append(ins)
            if changed:
                bb.instructions = new
    return nsplit


def build_nc(legalize=True):
    nc = bass.Bass("TRN2", debug=False, num_devices=NCORES)
    t_ext_d = nc.dram_tensor("t_ext", [SPC, T + 2], F32, kind="ExternalInput")
    p_rev_d = nc.dram_tensor("p_rev_ext", [SPC, T + 2], F32, kind="ExternalInput")
    out_d = nc.dram_tensor("out", [SPC, 2], F32, kind="ExternalOutput")

    with TileContext(nc) as tc:
        with tc.tile_pool(name="main", bufs=1) as pool:
            v = nc.vector
            s = nc.scalar

            # ---- persistent state ----
            R = pool.tile([SPC, ND * DL], F32, tag="R")
            text = pool.tile([SPC, T + 2], F32, tag="text")
            prev = pool.tile([SPC, T + 2], F32, tag="prev")
            sqi = pool.tile([SPC, 512], I32, tag="sqi")
            sqt = pool.tile([SPC, 512], F32, tag="sqt")
            Sr = pool.tile([SPC, KS * DL], F32, tag="Sr")
            Er = pool.tile([SPC, KE * DL], F32, tag="Er")
            acc = pool.tile([SPC, 1], F32, tag="acc")
            outp = pool.tile([SPC, 2], F32, tag="outp")

            nc.sync.dma_start(text[:, :], t_ext_d[:, :])
            nc.sync.dma_start(prev[:, :], p_rev_d[:, :])

            # sq table: sqt[col] = (col - SQOFF)^2, same in every partition
            nc.gpsimd.iota(sqi[:, :], pattern=[[1, 512]], base=0, channel_multiplier=0)
            nbias = pool.tile([SPC, 1], F32, tag="nbias")
            nc.gpsimd.memset(nbias[:, :], float(-SQOFF))
            s.activation(sqt[:, :], sqi[:, :], AF.Square, bias=nbias[:, 0:1])

            # ---- R boundary init (only slots ever read as BIG) ----
            # diag 0: positions 1..N+1 BIG, R[0][0] = 0 (disjoint writes)
            v.memset(R[:, 1 : N + 2], BIG)
            v.memset(R[:, DL : DL + N + 2], BIG)  # diag 1
            v.memset(R[:, 0:1], 0.0)
            # column 0 of diags 2..N+1 (lo-1 boundary, lower half)
            v.memset(R[:, 2 * DL : (N + 2) * DL : DL], BIG)
            # lo-1 boundary, upper half: diag d >= N+2 at position d-N-1
            v.memset(R[:, (N + 2) * DL + 1 : ND * DL : DL + 1], BIG)
            # hi+1 boundary, lower half: diag d in 2..N at position d
            v.memset(R[:, 2 * (DL + 1) : (N + 1) * (DL + 1) : DL + 1], BIG)
            # hi+1 boundary, upper half: diag d >= N+1 at position N+1
            v.memset(R[:, (N + 1) * DL + N + 1 : ND * DL : DL], BIG)

            v.memset(Sr[:, :], -BIG)
            # E ring: all zeros except E[2N][N] = 1 (disjoint writes — memset
            # accesses are not range-tracked, overlapping ones can reorder)
            e1 = ((2 * N) % KE) * DL + N
            v.memset(Er[:, 0:e1], 0.0)
            v.memset(Er[:, e1 : e1 + 1], 1.0)
            v.memset(Er[:, e1 + 1 : KE * DL], 0.0)
            v.memset(acc[:, :], 0.0)
            # scheduler fence: the R/ring init memsets above must not be
            # reordered past the DP steps (the range tracker misses some
            # small/strided overlaps)
            tc.no_sync_barrier()

            # ---- forward wavefront ----
            for d in range(2, 2 * N + 1):
                lo, hi = _rng(d)
                L = hi - lo + 1
                rb = d * DL
                p2s = R[:, (d - 2) * DL + lo - 1 : (d - 2) * DL + lo - 1 + L]
                p1s = R[:, (d - 1) * DL + lo - 1 : (d - 1) * DL + lo - 1 + L]
                p1 = R[:, (d - 1) * DL + lo : (d - 1) * DL + lo + L]

                m1 = pool.tile([SPC, DL], F32, tag="f_m1", bufs=3)
                mm = pool.tile([SPC, DL], F32, tag="f_mm", bufs=3)
                stk = pool.tile([SPC, 3 * DL], F32, tag="f_stk", bufs=3)
                est = pool.tile([SPC, 3 * DL], F32, tag="f_est", bufs=3)
                ssm = pool.tile([SPC, DL], F32, tag="f_ssm", bufs=3)
                lnb = pool.tile([SPC, DL], F32, tag="f_lnb", bufs=3)
                ds = pool.tile([SPC, DL], F32, tag="f_ds", bufs=3)
                dsq = pool.tile([SPC, DL], F32, tag="f_dsq", bufs=3)
                dm = pool.tile([SPC, DL], F32, tag="f_dm", bufs=3)

                v.tensor_tensor(m1[:, 0:L], p2s, p1s, op=OP.min)
                v.tensor_tensor(mm[:, 0:L], m1[:, 0:L], p1, op=OP.min)
                v.tensor_sub(stk[:, 0:L], p2s, mm[:, 0:L])
                v.tensor_sub(stk[:, L : 2 * L], p1s, mm[:, 0:L])
                v.tensor_sub(stk[:, 2 * L : 3 * L], p1, mm[:, 0:L])
                s.activation(est[:, 0 : 3 * L], stk[:, 0 : 3 * L], AF.Exp, scale=-INVG)
                v.tensor_reduce(
                    ssm[:, 0:L],
                    est[:, 0 : 3 * L].rearrange("p (a b) -> p b a", a=3),
                    axis=mybir.AxisListType.X,
                    op=OP.add,
                )
                s.activation(lnb[:, 0:L], ssm[:, 0:L], AF.Ln)
                # D diagonal: (t[idx-1] - p[d-idx-1])^2 via padded/reversed reads
                v.tensor_sub(
                    ds[:, 0:L],
                    text[:, lo : lo + L],
                    prev[:, N - d + lo + 1 : N - d + lo + 1 + L],
                )
                s.activation(dsq[:, 0:L], ds[:, 0:L], AF.Square)
                v.tensor_add(dm[:, 0:L], dsq[:, 0:L], mm[:, 0:L])
                v.scalar_tensor_tensor(
                    R[:, rb + lo : rb + lo + L],
                    lnb[:, 0:L],
                    -GAMMA,
                    dm[:, 0:L],
                    op0=OP.mult,
                    op1=OP.add,
                )

            # ---- backward (E recursion + Omega accumulation) ----
            def s_prep(dd):
                """S[dd] = R[dd] - D[dd] over extended range [lo-1, hi+1]."""
                lo, hi = _rng(dd)
                elo, ehi = lo - 1, hi + 1
                EL = ehi - elo + 1
                sb = (dd % KS) * DL
                ds2 = pool.tile([SPC, DL], F32, tag="b_ds2", bufs=KS)
                dq2 = pool.tile([SPC, DL], F32, tag="b_dq2", bufs=KS)
                v.tensor_sub(
                    ds2[:, 0:EL],
                    text[:, elo : elo + EL],
                    prev[:, N - dd + elo + 1 : N - dd + elo + 1 + EL],
                )
                s.activation(dq2[:, 0:EL], ds2[:, 0:EL], AF.Square)
                v.tensor_sub(
                    Sr[:, sb + elo : sb + elo + EL],
                    R[:, dd * DL + elo : dd * DL + elo + EL],
                    dq2[:, 0:EL],
                )

            s_prep(2 * N)

            for d in range(2 * N - 1, 1, -1):
                lo, hi = _rng(d)
                L = hi - lo + 1
                if d + 1 < 2 * N:
                    s_prep(d + 1)
                S1 = Sr[:, ((d + 1) % KS) * DL : ((d + 1) % KS) * DL + DL]
                S2 = Sr[:, ((d + 2) % KS) * DL : ((d + 2) % KS) * DL + DL]
                E1 = Er[:, ((d + 1) % KE) * DL : ((d + 1) % KE) * DL + DL]
                E2 = Er[:, ((d + 2) % KE) * DL : ((d + 2) % KE) * DL + DL]
                Ed = Er[:, (d % KE) * DL : (d % KE) * DL + DL]
                Rd = R[:, d * DL + lo : d * DL + lo + L]

                bst = pool.tile([SPC, 3 * DL], F32, tag="b_bst", bufs=3)
                bes = pool.tile([SPC, 3 * DL], F32, tag="b_bes", bufs=3)
                pst = pool.tile([SPC, 3 * DL], F32, tag="b_pst", bufs=3)
                scr = pool.tile([SPC, DL], F32, tag="b_scr", bufs=3)

                v.tensor_sub(bst[:, 0:L], S1[:, lo + 1 : lo + 1 + L], Rd)
                v.tensor_sub(bst[:, L : 2 * L], S1[:, lo : lo + L], Rd)
                v.tensor_sub(bst[:, 2 * L : 3 * L], S2[:, lo + 1 : lo + 1 + L], Rd)
                s.activation(bes[:, 0 : 3 * L], bst[:, 0 : 3 * L], AF.Exp, scale=INVG)
                v.tensor_mul(pst[:, 0:L], bes[:, 0:L], E1[:, lo + 1 : lo + 1 + L])
                v.tensor_mul(pst[:, L : 2 * L], bes[:, L : 2 * L], E1[:, lo : lo + L])
                v.tensor_mul(
                    pst[:, 2 * L : 3 * L], bes[:, 2 * L : 3 * L], E2[:, lo + 1 : lo + 1 + L]
                )
                v.tensor_reduce(
                    Ed[:, lo : lo + L],
                    pst[:, 0 : 3 * L].rearrange("p (a b) -> p b a", a=3),
                    axis=mybir.AxisListType.X,
                    op=OP.add,
                )
                # Omega: weight (2*idx - d)^2 = sqtab read at stride 2;
                # STT out = Ed * sqt, accum_out = per-partition sum
                c0 = 2 * lo - d + SQOFF
                stp = pool.tile([SPC, 1], F32, tag="b_stp", bufs=3)
                v.scalar_tensor_tensor(
                    scr[:, 0:L],
                    Ed[:, lo : lo + L],
                    1.0,
                    sqt[:, c0 : c0 + 2 * L : 2],
                    op0=OP.bypass,
                    op1=OP.mult,
                    accum_out=stp[:, 0:1],
                )
                v.tensor_add(acc[:, 0:1], acc[:, 0:1], stp[:, 0:1])

            v.tensor_copy(outp[:, 0:1], R[:, 2 * N * DL + N : 2 * N * DL + N + 1])
            v.tensor_copy(outp[:, 1:2], acc[:, 0:1])
            nc.sync.dma_start(out_d[:, :], outp[:, :])

    if legalize:
        _split_multi_waits(nc)
    return nc


def _shard_inputs(input, target):
    p = np.transpose(np.asarray(input, np.float32), (0, 2, 3, 4, 1)).reshape(-1, T)
    t = np.transpose(np.asarray(target, np.float32), (0, 2, 3, 4, 1)).reshape(-1, T)
    in_maps = []
    for k in range(NCORES):
        sl = slice(k * SPC, (k + 1) * SPC)
        t_ext = np.full((SPC, T + 2), SENT, np.float32)
        t_ext[:, 1 : T + 1] = t[sl]
        p_rev = np.full((SPC, T + 2), SENT, np.float32)
        p_rev[:, 1 : T + 1] = p[sl][:, ::-1]
        in_maps.append({"t_ext": t_ext, "p_rev_ext": p_rev})
    return in_maps


def _reduce_outputs(results):
    ls = np.concatenate([r["out"][:, 0] for r in results])
    tacc = np.concatenate([r["out"][:, 1] for r in results])
    loss_shape = ls.mean(dtype=np.float64)
    loss_temporal = (tacc / (T * T)).mean(dtype=np.float64)
    loss = ALPHA * loss_shape + (1.0 - ALPHA) * loss_temporal
    return np.array([loss, loss_shape, loss_temporal], np.float32)


def kernel(input, target, _cache={}):
    if "nc" not in _cache:
        _cache["nc"] = build_nc()
    res = bass_utils.run_bass_kernel_spmd(
        _cache["nc"], _shard_inputs(input, target), core_ids=list(range(NCORES))
    )
    return _reduce_outputs(res.results)


if __name__ == "__main__":
    key_inputs = None
